# revision 4
# baseline (speedup 1.0000x reference)
"""Trainium2 Bass kernel for nn_AttentionBlock (B=8, C=128, H=W=64).

Data-parallel over batch across 8 NeuronCores (one batch element per core).
Per core, full 4096x4096 single-head attention:

  xt = x.T;  q = xt@(qw*scale) + qb*scale;  k = xt@kw;  v0 = xt@vw
  S = q k^T;  P = softmax(S);  out = xt + P@v0@pw + (vb@pw + pb)
  (k-bias is softmax-invariant; v-bias folds through rowsum==1)

Per-core dataflow:
  - fp16 on the PE everywhere (10-bit mantissa ~ tf32, full speed)
  - S row-tiles accumulate in PSUM in 1024-wide chunks; per chunk: DVE
    negated max (from PSUM) -> ScalarE exp with that bias straight from
    PSUM -> fp16 P + per-chunk rowsum (accum_out). No full-row PSUM
    residency, no S copy to SBUF.
  - per-chunk local-max corrections corr_h = exp(cm_h - rowmax) are
    folded into the P-transposes as diagonal moving operands
    (out = P_blk^T @ diag(corr)), and into the rowsum on the side.
  - P transposed on TensorE in 128x128 fp16 blocks -> PSUM, drained to
    SBUF by DVE/ACT (split), consumed by the PV accumulation.
  - proj on PE; H^T transposed back on TensorE; final fused
    scalar_tensor_tensor: out = H*(1/rowsum) + (xt + bias2) from PSUM.
"""

import numpy as np

C = 128
N = 4096  # tokens per batch element (64*64)
B = 8
H = W = 64

# how to apply the per-chunk softmax corrections:
#   "diag"   — diagonal moving operand in the PE transpose (free)
#   "dve"    — explicit DVE rescale of P chunks
#   "gpsimd" — explicit GPSIMD rescale of P chunks
CORR_MODE = "gpsimd"

_cache = {}


def _build(n_tokens=N, corr_mode=None):
    import concourse.bass as bass
    import concourse.mybir as mybir
    import concourse.tile as tile
    from concourse import bacc
    from concourse.masks import make_identity

    corr_mode = corr_mode or CORR_MODE
    f32 = mybir.dt.float32
    f16 = mybir.dt.float16
    Alu = mybir.AluOpType
    Act = mybir.ActivationFunctionType

    NTOK = n_tokens
    NTILES = NTOK // 128          # q-token row tiles
    MCHUNK = min(1024, NTOK)      # S psum chunk width (2 banks)
    MCH_CNT = NTOK // MCHUNK      # chunks per row-tile
    BPC = MCHUNK // 128           # 128-wide m blocks per chunk
    GRP = min(512, NTOK)          # PV n-group size
    TPG = GRP // 128              # row-tiles per group
    NGRP = NTOK // GRP
    MBLK = NTOK // 128            # m blocks total

    nc = bacc.Bacc("TRN2", target_bir_lowering=False, debug=False, num_devices=8)

    xh_e = nc.dram_tensor("xh", [C, NTOK], f16, kind="ExternalInput")
    xpb_e = nc.dram_tensor("xpb", [NTOK, C], f32, kind="ExternalInput")
    qw_e = nc.dram_tensor("qw", [C, C], f16, kind="ExternalInput")
    kw_e = nc.dram_tensor("kw", [C, C], f16, kind="ExternalInput")
    vw_e = nc.dram_tensor("vw", [C, C], f16, kind="ExternalInput")
    pw_e = nc.dram_tensor("pw", [C, C], f16, kind="ExternalInput")
    qb_e = nc.dram_tensor("qb", [C, 1], f32, kind="ExternalInput")
    out_e = nc.dram_tensor("out", [NTOK, C], f32, kind="ExternalOutput")

    with tile.TileContext(nc) as tc:
        with (
            tc.tile_pool(name="persist", bufs=1) as persist,
            tc.tile_pool(name="pp", bufs=6) as p_pool,
            tc.tile_pool(name="ptc", bufs=6) as ptc_pool,
            tc.tile_pool(name="dgp", bufs=3 * TPG * MCH_CNT) as dg_pool,
            tc.tile_pool(name="small", bufs=2) as small,
            tc.tile_pool(name="stats", bufs=6) as stats,
            tc.tile_pool(name="ivp", bufs=2 * TPG + 2) as ivp,
            tc.tile_pool(name="psA", bufs=1, space="PSUM") as psA,
            tc.tile_pool(name="psB", bufs=1, space="PSUM") as psB,
            tc.tile_pool(name="psS", bufs=2, space="PSUM") as psS,
            tc.tile_pool(name="psT", bufs=2, space="PSUM") as psT,
        ):
            # ---- constants / weights ----
            qw_sb = persist.tile([C, C], f16, tag="qw")
            kw_sb = persist.tile([C, C], f16, tag="kw")
            vw_sb = persist.tile([C, C], f16, tag="vw")
            pw_sb = persist.tile([C, C], f16, tag="pw")
            qb_sb = persist.tile([C, 1], f32, tag="qb")
            nc.gpsimd.dma_start(out=qw_sb[:], in_=qw_e[:])
            nc.gpsimd.dma_start(out=kw_sb[:], in_=kw_e[:])
            nc.gpsimd.dma_start(out=vw_sb[:], in_=vw_e[:])
            nc.gpsimd.dma_start(out=pw_sb[:], in_=pw_e[:])
            nc.gpsimd.dma_start(out=qb_sb[:], in_=qb_e[:])

            ident = persist.tile([C, C], f16, tag="ident")
            make_identity(nc, ident[:])

            xh_sb = persist.tile([C, NTOK], f16, tag="xh")
            nc.gpsimd.dma_start(out=xh_sb[:], in_=xh_e[:])

            # ---- QT / KT (c_out, n) fp16 ----
            QT = persist.tile([C, NTOK], f16, tag="QT")
            KT = persist.tile([C, NTOK], f16, tag="KT")
            for j in range(NTOK // 512):
                sl = slice(j * 512, (j + 1) * 512)
                pq = psA.tile([C, 512], f32, tag="a")
                nc.tensor.matmul(pq[:], lhsT=qw_sb[:], rhs=xh_sb[:, sl])
                nc.vector.tensor_scalar(
                    out=QT[:, sl], in0=pq[:], scalar1=qb_sb[:], scalar2=None,
                    op0=Alu.add,
                )
                pk = psB.tile([C, 512], f32, tag="b")
                nc.tensor.matmul(pk[:], lhsT=kw_sb[:], rhs=xh_sb[:, sl])
                nc.scalar.activation(out=KT[:, sl], in_=pk[:], func=Act.Copy)

            # ---- V in (m, c) layout: V[i*128+p, c] at V_sb[p, i, c] ----
            V_sb = persist.tile([C, MBLK, 128], f16, tag="V")
            for i in range(MBLK):
                pv = psB.tile([C, 512], f32, tag="b")
                nc.tensor.matmul(
                    pv[:, :128], lhsT=xh_sb[:, i * 128:(i + 1) * 128],
                    rhs=vw_sb[:],
                )
                nc.scalar.activation(out=V_sb[:, i, :], in_=pv[:, :128],
                                     func=Act.Copy)

            iv_tiles = [None] * NTILES

            for g in range(NGRP):
                # ---- S + per-chunk softmax for this group's row-tiles ----
                P_tiles = []
                dg_tiles = []
                for t in range(TPG):
                    nt = g * TPG + t
                    qsl = slice(nt * 128, (nt + 1) * 128)
                    nm = stats.tile([C, MCH_CNT], f32, tag="nm")
                    rsc = stats.tile([C, MCH_CNT], f32, tag="rsc")
                    P_t = p_pool.tile([C, NTOK], f16, tag="P")
                    for h in range(MCH_CNT):
                        sps = psS.tile([C, MCHUNK], f32, tag="s")
                        for q in range(MCHUNK // 512):
                            nc.tensor.matmul(
                                sps[:, q * 512:(q + 1) * 512],
                                lhsT=QT[:, qsl],
                                rhs=KT[:, h * MCHUNK + q * 512:
                                       h * MCHUNK + (q + 1) * 512],
                            )
                        nc.vector.tensor_reduce(
                            out=nm[:, h:h + 1], in_=sps[:],
                            axis=mybir.AxisListType.X, op=Alu.max,
                            negate=True,
                        )
                        nc.scalar.activation(
                            out=P_t[:, h * MCHUNK:(h + 1) * MCHUNK],
                            in_=sps[:], func=Act.Exp,
                            bias=nm[:, h:h + 1], scale=1.0,
                            accum_out=rsc[:, h:h + 1],
                        )
                    # global row max M = -min(nm); corr_h = exp(cm_h - M)
                    ngm = stats.tile([C, 1], f32, tag="ngm")
                    nc.vector.tensor_reduce(
                        out=ngm[:], in_=nm[:], axis=mybir.AxisListType.X,
                        op=Alu.min,
                    )
                    corrs = stats.tile([C, MCH_CNT], f32, tag="corrs")
                    nc.scalar.activation(
                        out=corrs[:], in_=nm[:], func=Act.Exp,
                        bias=ngm[:], scale=-1.0,
                    )
                    # rowsum = sum_h rsc_h * corr_h ; iv = 1/rowsum
                    rsm = stats.tile([C, MCH_CNT], f32, tag="rsm")
                    nc.vector.tensor_tensor(
                        out=rsm[:], in0=rsc[:], in1=corrs[:], op=Alu.mult,
                    )
                    rs = stats.tile([C, 1], f32, tag="rs")
                    nc.vector.tensor_reduce(
                        out=rs[:], in_=rsm[:], axis=mybir.AxisListType.X,
                        op=Alu.add,
                    )
                    iv = ivp.tile([C, 1], f32, tag="iv")
                    nc.vector.reciprocal(iv[:], rs[:])
                    iv_tiles[nt] = iv
                    P_tiles.append(P_t)

                    if corr_mode == "diag":
                        dgs = []
                        for h in range(MCH_CNT):
                            dg = dg_pool.tile([C, C], f16, tag="dg")
                            nc.vector.tensor_scalar(
                                out=dg[:], in0=ident[:],
                                scalar1=corrs[:, h:h + 1], scalar2=None,
                                op0=Alu.mult,
                            )
                            dgs.append(dg)
                        dg_tiles.append(dgs)
                    else:
                        eng = nc.vector if corr_mode == "dve" else nc.gpsimd
                        for h in range(MCH_CNT):
                            eng.tensor_scalar(
                                out=P_t[:, h * MCHUNK:(h + 1) * MCHUNK],
                                in0=P_t[:, h * MCHUNK:(h + 1) * MCHUNK],
                                scalar1=corrs[:, h:h + 1], scalar2=None,
                                op0=Alu.mult,
                            )

                # ---- PE-transpose P blocks (w/ diag), PV accumulation ----
                O_ps = psA.tile([C, GRP], f32, tag="a")
                for i in range(MBLK):
                    tp = psT.tile([C, GRP], f16, tag="t")
                    for t in range(TPG):
                        mov = (dg_tiles[t][i // BPC][:]
                               if corr_mode == "diag" else ident[:])
                        nc.tensor.transpose(
                            tp[:, t * 128:(t + 1) * 128],
                            P_tiles[t][:, i * 128:(i + 1) * 128],
                            mov,
                        )
                    ptc = ptc_pool.tile([C, GRP], f16, tag="pt")
                    if i % 2 == 0:
                        nc.vector.tensor_copy(ptc[:], tp[:])
                    else:
                        nc.scalar.activation(out=ptc[:], in_=tp[:],
                                             func=Act.Copy)
                    nc.tensor.matmul(
                        O_ps[:], lhsT=V_sb[:, i, :], rhs=ptc[:],
                        start=(i == 0), stop=(i == MBLK - 1),
                    )
                O_sb = small.tile([C, GRP], f16, tag="O")
                nc.scalar.activation(out=O_sb[:], in_=O_ps[:], func=Act.Copy)

                # ---- proj, transpose back, residual+bias+normalize ----
                H_ps = psB.tile([C, GRP], f32, tag="b")
                nc.tensor.matmul(H_ps[:], lhsT=pw_sb[:], rhs=O_sb[:])
                Hs = small.tile([C, GRP], f16, tag="Hs")
                nc.scalar.activation(out=Hs[:], in_=H_ps[:], func=Act.Copy)

                tph = psT.tile([C, GRP], f16, tag="t")
                for t in range(TPG):
                    nc.tensor.transpose(
                        tph[:, t * 128:(t + 1) * 128],
                        Hs[:, t * 128:(t + 1) * 128], ident[:],
                    )

                xpb_g = small.tile([C, TPG, 128], f32, tag="xpb")
                nc.gpsimd.dma_start(
                    out=xpb_g[:],
                    in_=xpb_e[g * GRP:(g + 1) * GRP, :].rearrange(
                        "(t p) c -> p t c", p=128),
                )
                out_g = small.tile([C, TPG, 128], f32, tag="og")
                for t in range(TPG):
                    nt = g * TPG + t
                    nc.vector.scalar_tensor_tensor(
                        out=out_g[:, t, :],
                        in0=tph[:, t * 128:(t + 1) * 128],
                        scalar=iv_tiles[nt][:], in1=xpb_g[:, t, :],
                        op0=Alu.mult, op1=Alu.add,
                    )
                nc.gpsimd.dma_start(
                    out=out_e[g * GRP:(g + 1) * GRP, :].rearrange(
                        "(t p) c -> p t c", p=128),
                    in_=out_g[:],
                )

    nc.compile()
    return nc


def _get_nc(n_tokens=N):
    if n_tokens not in _cache:
        _cache[n_tokens] = _build(n_tokens)
    return _cache[n_tokens]


def prep_inputs(x, qw, qb, kw, kb, vw, vb, proj_w, proj_b, n_tokens=N):
    """Host-side prep: shard over batch, fold scale/biases, transpose."""
    x = np.asarray(x, dtype=np.float32)
    b, c, h, w = x.shape
    scale = c ** (-0.5)
    qw_s = (np.asarray(qw, np.float32) * scale).astype(np.float16)
    kw16 = np.asarray(kw, np.float32).astype(np.float16)
    vw16 = np.asarray(vw, np.float32).astype(np.float16)
    pw16 = np.asarray(proj_w, np.float32).astype(np.float16)
    qb_s = (np.asarray(qb, np.float32) * scale).reshape(c, 1).astype(np.float32)
    pb2 = (np.asarray(vb, np.float32) @ np.asarray(proj_w, np.float32)
           + np.asarray(proj_b, np.float32)).astype(np.float32)

    in_maps = []
    for i in range(b):
        xc = x[i].reshape(c, h * w)[:, :n_tokens]
        xt = xc.T.copy()
        in_maps.append({
            "xh": np.ascontiguousarray(xc).astype(np.float16),
            "xpb": np.ascontiguousarray(xt + pb2[None, :]),
            "qw": qw_s, "kw": kw16, "vw": vw16, "pw": pw16,
            "qb": qb_s,
        })
    return in_maps


def kernel(x, qw, qb, kw, kb, vw, vb, proj_w, proj_b, _trace=False):
    from concourse.bass_utils import run_bass_kernel_spmd

    nc = _get_nc(N)
    in_maps = prep_inputs(x, qw, qb, kw, kb, vw, vb, proj_w, proj_b)
    res = run_bass_kernel_spmd(nc, in_maps, core_ids=list(range(B)),
                               trace=_trace)
    kernel.last_results = res
    out = np.stack([np.asarray(res.results[i]["out"]) for i in range(B)])
    return out.reshape(B, H, W, C).astype(np.float32)


# revision 5
# speedup vs baseline: 4.7918x; 4.7918x over previous
"""Trainium2 Bass kernel for nn_AttentionBlock (B=8, C=128, H=W=64).

Data-parallel over batch across 8 NeuronCores (one batch element per core).
Per core, full 4096x4096 single-head attention:

  xt = x.T;  q = xt@(qw*scale) + qb*scale;  k = xt@kw;  v0 = xt@vw
  S = q k^T;  P = softmax(S);  out = xt + P@v0@pw + (vb@pw + pb)
  (k-bias is softmax-invariant; v-bias folds through rowsum==1)

Per-core dataflow:
  - fp16 on the PE everywhere (10-bit mantissa ~ tf32, full speed)
  - S row-tiles accumulate in PSUM in 1024-wide chunks; per chunk: DVE
    negated max (from PSUM) -> ScalarE exp with that bias straight from
    PSUM -> fp16 P + per-chunk rowsum (accum_out). No full-row PSUM
    residency, no S copy to SBUF.
  - per-chunk local-max corrections corr_h = exp(cm_h - rowmax) are
    folded into the P-transposes as diagonal moving operands
    (out = P_blk^T @ diag(corr)), and into the rowsum on the side.
  - P transposed on TensorE in 128x128 fp16 blocks -> PSUM, drained to
    SBUF by DVE/ACT (split), consumed by the PV accumulation.
  - proj on PE; H^T transposed back on TensorE; final fused
    scalar_tensor_tensor: out = H*(1/rowsum) + (xt + bias2) from PSUM.
"""

import numpy as np

C = 128
N = 4096  # tokens per batch element (64*64)
B = 8
H = W = 64

# how to apply the per-chunk softmax corrections:
#   "diag"   — diagonal moving operand in the PE transpose (free)
#   "dve"    — explicit DVE rescale of P chunks
#   "gpsimd" — explicit GPSIMD rescale of P chunks
CORR_MODE = "dve"

_cache = {}


def _build(n_tokens=N, corr_mode=None):
    import concourse.bass as bass
    import concourse.mybir as mybir
    import concourse.tile as tile
    from concourse import bacc
    from concourse.masks import make_identity

    corr_mode = corr_mode or CORR_MODE
    f32 = mybir.dt.float32
    f16 = mybir.dt.float16
    Alu = mybir.AluOpType
    Act = mybir.ActivationFunctionType

    NTOK = n_tokens
    NTILES = NTOK // 128          # q-token row tiles
    MCHUNK = min(1024, NTOK)      # S psum chunk width (2 banks)
    MCH_CNT = NTOK // MCHUNK      # chunks per row-tile
    BPC = MCHUNK // 128           # 128-wide m blocks per chunk
    GRP = min(512, NTOK)          # PV n-group size
    TPG = GRP // 128              # row-tiles per group
    NGRP = NTOK // GRP
    MBLK = NTOK // 128            # m blocks total

    nc = bacc.Bacc("TRN2", target_bir_lowering=False, debug=False, num_devices=8)

    xh_e = nc.dram_tensor("xh", [C, NTOK], f16, kind="ExternalInput")
    xpb_e = nc.dram_tensor("xpb", [NTOK, C], f32, kind="ExternalInput")
    qw_e = nc.dram_tensor("qw", [C, C], f16, kind="ExternalInput")
    kw_e = nc.dram_tensor("kw", [C, C], f16, kind="ExternalInput")
    vw_e = nc.dram_tensor("vw", [C, C], f16, kind="ExternalInput")
    pw_e = nc.dram_tensor("pw", [C, C], f16, kind="ExternalInput")
    qb_e = nc.dram_tensor("qb", [C, 1], f32, kind="ExternalInput")
    out_e = nc.dram_tensor("out", [NTOK, C], f32, kind="ExternalOutput")

    with tile.TileContext(nc) as tc:
        with (
            tc.tile_pool(name="persist", bufs=1) as persist,
            tc.tile_pool(name="pp", bufs=6) as p_pool,
            tc.tile_pool(name="ptc", bufs=6) as ptc_pool,
            tc.tile_pool(name="dgp", bufs=3 * TPG * MCH_CNT) as dg_pool,
            tc.tile_pool(name="small", bufs=2) as small,
            tc.tile_pool(name="stats", bufs=6) as stats,
            tc.tile_pool(name="ivp", bufs=2 * TPG + 2) as ivp,
            tc.tile_pool(name="psA", bufs=1, space="PSUM") as psA,
            tc.tile_pool(name="psB", bufs=1, space="PSUM") as psB,
            tc.tile_pool(name="psS", bufs=2, space="PSUM") as psS,
            tc.tile_pool(name="psT", bufs=2, space="PSUM") as psT,
        ):
            # ---- constants / weights ----
            qw_sb = persist.tile([C, C], f16, tag="qw")
            kw_sb = persist.tile([C, C], f16, tag="kw")
            vw_sb = persist.tile([C, C], f16, tag="vw")
            pw_sb = persist.tile([C, C], f16, tag="pw")
            qb_sb = persist.tile([C, 1], f32, tag="qb")
            nc.gpsimd.dma_start(out=qw_sb[:], in_=qw_e[:])
            nc.gpsimd.dma_start(out=kw_sb[:], in_=kw_e[:])
            nc.gpsimd.dma_start(out=vw_sb[:], in_=vw_e[:])
            nc.gpsimd.dma_start(out=pw_sb[:], in_=pw_e[:])
            nc.gpsimd.dma_start(out=qb_sb[:], in_=qb_e[:])

            ident = persist.tile([C, C], f16, tag="ident")
            make_identity(nc, ident[:])

            xh_sb = persist.tile([C, NTOK], f16, tag="xh")
            nc.gpsimd.dma_start(out=xh_sb[:], in_=xh_e[:])

            # ---- QT / KT (c_out, n) fp16 ----
            QT = persist.tile([C, NTOK], f16, tag="QT")
            KT = persist.tile([C, NTOK], f16, tag="KT")
            for j in range(NTOK // 512):
                sl = slice(j * 512, (j + 1) * 512)
                pq = psA.tile([C, 512], f32, tag="a")
                nc.tensor.matmul(pq[:], lhsT=qw_sb[:], rhs=xh_sb[:, sl])
                nc.vector.tensor_scalar(
                    out=QT[:, sl], in0=pq[:], scalar1=qb_sb[:], scalar2=None,
                    op0=Alu.add,
                )
                pk = psB.tile([C, 512], f32, tag="b")
                nc.tensor.matmul(pk[:], lhsT=kw_sb[:], rhs=xh_sb[:, sl])
                nc.scalar.activation(out=KT[:, sl], in_=pk[:], func=Act.Copy)

            # ---- V in (m, c) layout: V[i*128+p, c] at V_sb[p, i, c] ----
            V_sb = persist.tile([C, MBLK, 128], f16, tag="V")
            for i in range(MBLK):
                pv = psB.tile([C, 512], f32, tag="b")
                nc.tensor.matmul(
                    pv[:, :128], lhsT=xh_sb[:, i * 128:(i + 1) * 128],
                    rhs=vw_sb[:],
                )
                nc.scalar.activation(out=V_sb[:, i, :], in_=pv[:, :128],
                                     func=Act.Copy)

            iv_tiles = [None] * NTILES

            for g in range(NGRP):
                # ---- S + per-chunk softmax for this group's row-tiles ----
                P_tiles = []
                dg_tiles = []
                for t in range(TPG):
                    nt = g * TPG + t
                    qsl = slice(nt * 128, (nt + 1) * 128)
                    nm = stats.tile([C, MCH_CNT], f32, tag="nm")
                    rsc = stats.tile([C, MCH_CNT], f32, tag="rsc")
                    P_t = p_pool.tile([C, NTOK], f16, tag="P")
                    for h in range(MCH_CNT):
                        sps = psS.tile([C, MCHUNK], f32, tag="s")
                        for q in range(MCHUNK // 512):
                            nc.tensor.matmul(
                                sps[:, q * 512:(q + 1) * 512],
                                lhsT=QT[:, qsl],
                                rhs=KT[:, h * MCHUNK + q * 512:
                                       h * MCHUNK + (q + 1) * 512],
                            )
                        nc.vector.tensor_reduce(
                            out=nm[:, h:h + 1], in_=sps[:],
                            axis=mybir.AxisListType.X, op=Alu.max,
                            negate=True,
                        )
                        nc.scalar.activation(
                            out=P_t[:, h * MCHUNK:(h + 1) * MCHUNK],
                            in_=sps[:], func=Act.Exp,
                            bias=nm[:, h:h + 1], scale=1.0,
                            accum_out=rsc[:, h:h + 1],
                        )
                    # global row max M = -min(nm); corr_h = exp(cm_h - M)
                    ngm = stats.tile([C, 1], f32, tag="ngm")
                    nc.vector.tensor_reduce(
                        out=ngm[:], in_=nm[:], axis=mybir.AxisListType.X,
                        op=Alu.min,
                    )
                    corrs = stats.tile([C, MCH_CNT], f32, tag="corrs")
                    nc.scalar.activation(
                        out=corrs[:], in_=nm[:], func=Act.Exp,
                        bias=ngm[:], scale=-1.0,
                    )
                    # rowsum = sum_h rsc_h * corr_h ; iv = 1/rowsum
                    rsm = stats.tile([C, MCH_CNT], f32, tag="rsm")
                    nc.vector.tensor_tensor(
                        out=rsm[:], in0=rsc[:], in1=corrs[:], op=Alu.mult,
                    )
                    rs = stats.tile([C, 1], f32, tag="rs")
                    nc.vector.tensor_reduce(
                        out=rs[:], in_=rsm[:], axis=mybir.AxisListType.X,
                        op=Alu.add,
                    )
                    iv = ivp.tile([C, 1], f32, tag="iv")
                    nc.vector.reciprocal(iv[:], rs[:])
                    iv_tiles[nt] = iv
                    P_tiles.append(P_t)

                    if corr_mode == "diag":
                        dgs = []
                        for h in range(MCH_CNT):
                            dg = dg_pool.tile([C, C], f16, tag="dg")
                            nc.vector.tensor_scalar(
                                out=dg[:], in0=ident[:],
                                scalar1=corrs[:, h:h + 1], scalar2=None,
                                op0=Alu.mult,
                            )
                            dgs.append(dg)
                        dg_tiles.append(dgs)
                    else:
                        eng = nc.vector if corr_mode == "dve" else nc.gpsimd
                        for h in range(MCH_CNT):
                            eng.tensor_scalar(
                                out=P_t[:, h * MCHUNK:(h + 1) * MCHUNK],
                                in0=P_t[:, h * MCHUNK:(h + 1) * MCHUNK],
                                scalar1=corrs[:, h:h + 1], scalar2=None,
                                op0=Alu.mult,
                            )

                # ---- PE-transpose P blocks (w/ diag), PV accumulation ----
                O_ps = psA.tile([C, GRP], f32, tag="a")
                for i in range(MBLK):
                    tp = psT.tile([C, GRP], f16, tag="t")
                    for t in range(TPG):
                        mov = (dg_tiles[t][i // BPC][:]
                               if corr_mode == "diag" else ident[:])
                        nc.tensor.transpose(
                            tp[:, t * 128:(t + 1) * 128],
                            P_tiles[t][:, i * 128:(i + 1) * 128],
                            mov,
                        )
                    ptc = ptc_pool.tile([C, GRP], f16, tag="pt")
                    if i % 2 == 0:
                        nc.vector.tensor_copy(ptc[:], tp[:])
                    else:
                        nc.scalar.activation(out=ptc[:], in_=tp[:],
                                             func=Act.Copy)
                    nc.tensor.matmul(
                        O_ps[:], lhsT=V_sb[:, i, :], rhs=ptc[:],
                        start=(i == 0), stop=(i == MBLK - 1),
                    )
                O_sb = small.tile([C, GRP], f16, tag="O")
                nc.scalar.activation(out=O_sb[:], in_=O_ps[:], func=Act.Copy)

                # ---- proj, transpose back, residual+bias+normalize ----
                H_ps = psB.tile([C, GRP], f32, tag="b")
                nc.tensor.matmul(H_ps[:], lhsT=pw_sb[:], rhs=O_sb[:])
                Hs = small.tile([C, GRP], f16, tag="Hs")
                nc.scalar.activation(out=Hs[:], in_=H_ps[:], func=Act.Copy)

                tph = psT.tile([C, GRP], f16, tag="t")
                for t in range(TPG):
                    nc.tensor.transpose(
                        tph[:, t * 128:(t + 1) * 128],
                        Hs[:, t * 128:(t + 1) * 128], ident[:],
                    )

                xpb_g = small.tile([C, TPG, 128], f32, tag="xpb")
                nc.gpsimd.dma_start(
                    out=xpb_g[:],
                    in_=xpb_e[g * GRP:(g + 1) * GRP, :].rearrange(
                        "(t p) c -> p t c", p=128),
                )
                out_g = small.tile([C, TPG, 128], f32, tag="og")
                for t in range(TPG):
                    nt = g * TPG + t
                    nc.vector.scalar_tensor_tensor(
                        out=out_g[:, t, :],
                        in0=tph[:, t * 128:(t + 1) * 128],
                        scalar=iv_tiles[nt][:], in1=xpb_g[:, t, :],
                        op0=Alu.mult, op1=Alu.add,
                    )
                nc.gpsimd.dma_start(
                    out=out_e[g * GRP:(g + 1) * GRP, :].rearrange(
                        "(t p) c -> p t c", p=128),
                    in_=out_g[:],
                )

    nc.compile()
    return nc


def _get_nc(n_tokens=N):
    if n_tokens not in _cache:
        _cache[n_tokens] = _build(n_tokens)
    return _cache[n_tokens]


def prep_inputs(x, qw, qb, kw, kb, vw, vb, proj_w, proj_b, n_tokens=N):
    """Host-side prep: shard over batch, fold scale/biases, transpose."""
    x = np.asarray(x, dtype=np.float32)
    b, c, h, w = x.shape
    scale = c ** (-0.5)
    qw_s = (np.asarray(qw, np.float32) * scale).astype(np.float16)
    kw16 = np.asarray(kw, np.float32).astype(np.float16)
    vw16 = np.asarray(vw, np.float32).astype(np.float16)
    pw16 = np.asarray(proj_w, np.float32).astype(np.float16)
    qb_s = (np.asarray(qb, np.float32) * scale).reshape(c, 1).astype(np.float32)
    pb2 = (np.asarray(vb, np.float32) @ np.asarray(proj_w, np.float32)
           + np.asarray(proj_b, np.float32)).astype(np.float32)

    in_maps = []
    for i in range(b):
        xc = x[i].reshape(c, h * w)[:, :n_tokens]
        xt = xc.T.copy()
        in_maps.append({
            "xh": np.ascontiguousarray(xc).astype(np.float16),
            "xpb": np.ascontiguousarray(xt + pb2[None, :]),
            "qw": qw_s, "kw": kw16, "vw": vw16, "pw": pw16,
            "qb": qb_s,
        })
    return in_maps


def kernel(x, qw, qb, kw, kb, vw, vb, proj_w, proj_b, _trace=False):
    from concourse.bass_utils import run_bass_kernel_spmd

    nc = _get_nc(N)
    in_maps = prep_inputs(x, qw, qb, kw, kb, vw, vb, proj_w, proj_b)
    res = run_bass_kernel_spmd(nc, in_maps, core_ids=list(range(B)),
                               trace=_trace)
    kernel.last_results = res
    out = np.stack([np.asarray(res.results[i]["out"]) for i in range(B)])
    return out.reshape(B, H, W, C).astype(np.float32)
